# revision 1
# baseline (speedup 1.0000x reference)
"""Bilateral anti-alias filter on Trainium2, 8-core data parallel.

Full inputs: images [16,3,512,512] f32, spatial_kernel [5,5] f32.
Shards the batch over 8 NeuronCores (2 images each), runs a Bass/Tile
kernel per core, gathers the full output.

Math (per pixel, K=5, sigma_i=0.1):
  w_t = s_t * exp(-50*(p_t - c)^2),  out = sum(w_t*p_t)/(sum(w_t)+eps)
Pair symmetry: w for tap -(di,dj) at pixel r equals w for tap +(di,dj)
at pixel r-(di,dj).  So only 12 weight planes are computed; each
contributes 4 terms (direct + shifted, for numerator + denominator),
accumulated on the TensorEngine via 0/1 shift-matrix matmuls into PSUM.
"""
import sys

sys.path.insert(0, "/opt/trn_rl_repo")

import numpy as np
import ml_dtypes
from contextlib import ExitStack

import concourse.bass as bass
import concourse.tile as tile
from concourse import bacc, mybir
from concourse.bass_utils import run_bass_kernel_spmd

f32 = mybir.dt.float32
bf16 = mybir.dt.bfloat16
AF = mybir.ActivationFunctionType
Alu = mybir.AluOpType

N_CORES = 8
B_FULL, C, H, W = 16, 3, 512, 512
B_SH = B_FULL // N_CORES  # 2 images per core
KK = 5
PAD = KK // 2  # 2
INV2SIG2 = 1.0 / (2.0 * 0.1 * 0.1)  # 50.0

# 12 pairs (di, dj) with di >= 0, lexicographically positive
PAIRS = [
    (0, 1), (0, 2),
    (1, -2), (1, -1), (1, 0), (1, 1), (1, 2),
    (2, -2), (2, -1), (2, 0), (2, 1), (2, 2),
]

GROUPS = [
    [(0, 1), (1, 0)], [(1, -1), (1, 1)], [(0, 2), (2, 0)],
    [(1, -2), (1, 2)], [(2, -1), (2, 1)], [(2, -2), (2, 2)],
]
SUB_BF16 = True   # compute d = p - c in bf16 (DVE 2x mode)
SQ_GPS = False     # squares on GpSimd instead of ACT
NOUT = 124  # output rows per tile
NG = 128    # grid partitions (= NOUT + 4)
WB = W + 4  # 516: padded col buffer, idx j <-> col j-2


def _row_bands(h):
    """Tile start rows: uniform NOUT-row bands; last band overlaps upward."""
    bands = list(range(0, h - NOUT + 1, NOUT))
    if bands[-1] != h - NOUT:
        bands.append(h - NOUT)
    return bands


def _reflect_runs(v0, v1, h):
    """Split virtual row range [v0, v1] into runs of physical rows.
    Returns list of (p_offset, phys_start, count, step) with step +-1."""
    runs = []
    v = v0
    while v <= v1:
        if v < 0:
            # rows v..min(-1, v1) -> physical -v .. (reversed, step -1)
            e = min(-1, v1)
            runs.append((v - v0, -v, e - v + 1, -1))
            v = e + 1
        elif v >= h:
            e = v1
            runs.append((v - v0, 2 * h - 2 - v, e - v + 1, -1))
            v = e + 1
        else:
            e = min(h - 1, v1)
            runs.append((v - v0, v, e - v + 1, 1))
            v = e + 1
    return runs


def build_bilateral(nc, biases, h=H, w=W, b_sh=B_SH, c=C):
    """Emit the per-core program into nc (a Bacc). biases[(di,dj)] = ln s."""
    wb = w + 4
    img_d = nc.dram_tensor("images", [b_sh, c, h, w], f32, kind="ExternalInput").ap()
    shifts_d = nc.dram_tensor("shifts", [NG, 5, NOUT], bf16, kind="ExternalInput").ap()
    out_d = nc.dram_tensor("out", [b_sh, c, h, w], f32, kind="ExternalOutput").ap()

    # const APs for activation biases
    for val in sorted(set(biases.values()) | {1.0}):
        key = (f32, val)
        if key not in nc.const_aps.aps:
            t = nc.alloc_sbuf_tensor(f"cbias-{val}", [128, 1], f32)
            nc.gpsimd.memset(t.ap(), val)
            nc.const_aps.aps[key] = t.ap()
    nc.all_engine_barrier()

    bands = _row_bands(h)

    with tile.TileContext(nc) as tc, ExitStack() as ctx:
        consts = ctx.enter_context(tc.tile_pool(name="consts", bufs=1))
        imgs_f = ctx.enter_context(tc.tile_pool(name="imgs_f", bufs=2))
        imgs_b = ctx.enter_context(tc.tile_pool(name="imgs_b", bufs=2))
        planes = ctx.enter_context(tc.tile_pool(name="planes", bufs=2))
        prods = ctx.enter_context(tc.tile_pool(name="prods", bufs=4))
        finals = ctx.enter_context(tc.tile_pool(name="finals", bufs=1))
        psums = ctx.enter_context(tc.tile_pool(name="psums", bufs=1, space="PSUM"))

        shifts = consts.tile([NG, 5, NOUT], bf16)
        nc.sync.dma_start(shifts[:], shifts_d[:])

        for bi in range(b_sh):
            for r0 in bands:
                # ---- load 3 row-shifted reflect-padded image copies ----
                ifs = []
                for s in range(3):
                    t = imgs_f.tile([NG, c, wb], f32, tag=f"i{s}f")
                    refl_rows = []
                    for (po, ps, cnt, step) in _reflect_runs(
                        r0 - 2 + s, r0 - 2 + s + NG - 1, h
                    ):
                        if step == 1:
                            src = img_d[bi, :, ps : ps + cnt, :]
                            nc.sync.dma_start(
                                t[po : po + cnt, :, 2 : 2 + w],
                                src.rearrange("c r n -> r c n"),
                            )
                        else:
                            for k in range(cnt):
                                refl_rows.append((po + k, ps - k))
                    # reflect rows duplicate rows already in the tile:
                    # single-partition SBUF self-copies (main cols)
                    for (p_dst, phys) in refl_rows:
                        p_src = phys - (r0 - 2 + s)
                        nc.sync.dma_start(
                            t[p_dst : p_dst + 1, :, 2 : 2 + w],
                            t[p_src : p_src + 1, :, 2 : 2 + w],
                        )
                    # reflect pad cols: in-tile duplicates, tiny GPSIMD copies
                    for (j, jsrc) in ((0, 4), (1, 3), (2 + w, w), (3 + w, w - 1)):
                        nc.gpsimd.tensor_copy(
                            t[:, :, j : j + 1], t[:, :, jsrc : jsrc + 1]
                        )
                    ifs.append(t)

                # bf16 copies: A (cast), B (A shifted 1 col, via DMA)
                ibA, ibB = [], []
                for s in range(3):
                    a = imgs_b.tile([NG, c, wb], bf16, tag=f"i{s}bA")
                    nc.vector.tensor_copy(a[:], ifs[s][:])
                    ibA.append(a)
                    b = imgs_b.tile([NG, c, wb], bf16, tag=f"i{s}bB")
                    nc.sync.dma_start(b[:, :, 0 : wb - 1], a[:, :, 1:wb])
                    ibB.append(b)

                # ---- PSUM accumulators (512-wide per channel = one 2KB
                # zero-region/bank per channel, required for start flags) ----
                pw = psums.tile([NOUT, c, 512], f32, tag="pw")
                pa = psums.tile([NOUT, c, 512], f32, tag="pa")

                n_mm = len(PAIRS) * 2  # mms per psum target per channel
                mm_i = 0
                for grp in GROUPS:
                    G = len(grp)
                    dg = planes.tile([NG, G * c, w + 2], bf16, tag="d")
                    for gi, (di, dj) in enumerate(grp):
                        cP = -2 if dj > 0 else 0
                        if dj % 2 == 0:
                            dsrc = ibA[di][:, :, cP + dj + 2 : cP + dj + 4 + w]
                        else:
                            dsrc = ibB[di][:, :, cP + dj + 1 : cP + dj + 3 + w]
                        nc.vector.tensor_tensor(
                            dg[:, gi * c : (gi + 1) * c, :], dsrc,
                            ibA[0][:, :, cP + 2 : cP + 4 + w], Alu.subtract,
                        )
                    sqg = planes.tile([NG, G * c, w + 2], f32, tag="sq")
                    nc.scalar.activation(sqg[:], dg[:], AF.Square)
                    wg = planes.tile([NG, G * c, w + 2], bf16, tag="w")
                    nc.scalar.activation(
                        wg[:], sqg[:], AF.Exp,
                        bias=biases[grp[0]], scale=-INV2SIG2,
                    )
                    for gi, (di, dj) in enumerate(grp):
                        cP = -2 if dj > 0 else 0
                        wp = wg[:, gi * c : (gi + 1) * c, :]
                        # Z = w * img(center rows),  Y = w * img(shifted)
                        zp = prods.tile([NG, c, w + 2], bf16, tag="z")
                        nc.vector.tensor_tensor(
                            zp[:], wp[:], ibA[0][:, :, cP + 2 : cP + 2 + w + 2], Alu.mult
                        )
                        yp = prods.tile([NG, c, w], bf16, tag="y")
                        if dj % 2 == 0:
                            ysrc = ibA[di][:, :, dj + 2 : dj + 2 + w]
                        else:
                            ysrc = ibB[di][:, :, dj + 1 : dj + 1 + w]
                        nc.vector.tensor_tensor(
                            yp[:], wp[:, :, -cP : -cP + w], ysrc, Alu.mult
                        )

                        # ---- 4 matmuls per channel ----
                        s_dir = 2
                        s_sh = 2 - di
                        first = mm_i == 0
                        last = mm_i == n_mm - 2  # this pair adds 2 mms per target
                        w_merged = dj == 0  # dir+shifted share the rhs window
    # (merged lhsT idx: di=1 -> 3, di=2 -> 4)
                        for ch in range(c):
                            nc.tensor.matmul(
                                pw[:, ch, 0:w],
                                shifts[:, (2 + di) if w_merged else s_dir, :],
                                wp[:, ch, -cP : -cP + w],
                                start=first,
                                stop=last if w_merged else False,
                            )
                            nc.tensor.matmul(
                                pa[:, ch, 0:w],
                                shifts[:, s_dir, :],
                                yp[:, ch, :],
                                start=first,
                                stop=False,
                            )
                        for ch in range(c):
                            if not w_merged:
                                nc.tensor.matmul(
                                    pw[:, ch, 0:w],
                                    shifts[:, s_sh, :],
                                    wp[:, ch, -dj - cP : -dj - cP + w],
                                    start=False,
                                    stop=last,
                                )
                            nc.tensor.matmul(
                                pa[:, ch, 0:w],
                                shifts[:, s_sh, :],
                                zp[:, ch, -dj - cP : -dj - cP + w],
                                start=False,
                                stop=last,
                            )
                        mm_i += 2

                # ---- finalize: out = (pa + center) * exp(-ln(pw + 1)) ----
                lnv = finals.tile([NOUT, c, w], f32, tag="lnv")
                nc.scalar.activation(lnv[:], pw[:, :, 0:w], AF.Ln, bias=1.0)
                rec = finals.tile([NOUT, c, w], f32, tag="rec")
                nc.scalar.activation(rec[:], lnv[:], AF.Exp, scale=-1.0)
                acct = finals.tile([NOUT, c, w], f32, tag="acct")
                nc.vector.tensor_tensor(
                    acct[:], pa[:, :, 0:w], ifs[2][0:NOUT, :, 2 : 2 + w], Alu.add
                )
                res = finals.tile([NOUT, c, w], f32, tag="res")
                nc.vector.tensor_tensor(res[:], acct[:], rec[:], Alu.mult)
                # overlap band: only write rows not already written by the
                # previous band (avoids DRAM WAW serialization)
                oo = 0 if r0 == bands[0] else max(0, prev_end - r0)
                nc.sync.dma_start(
                    out_d[bi, :, r0 + oo : r0 + NOUT, :].rearrange("c r n -> r c n"),
                    res[oo:NOUT],
                )
                prev_end = r0 + NOUT
    return nc


def _shift_mats():
    s = np.zeros((NG, 5, NOUT), dtype=ml_dtypes.bfloat16)
    for k in range(3):
        for m in range(NOUT):
            s[m + k, k, m] = 1.0
    # merged direct+shifted for dj=0 planes: idx 3 = L2+L1, idx 4 = L2+L0
    s[:, 3] = s[:, 2] + s[:, 1]
    s[:, 4] = s[:, 2] + s[:, 0]
    return s


def make_program(spatial_kernel):
    biases = {}
    for (di, dj) in PAIRS:
        v = float(np.float32(np.log(np.float32(spatial_kernel[2 + di, 2 + dj]))))
        biases[(di, dj)] = v
    nc = bacc.Bacc("TRN2", target_bir_lowering=False, debug=False)
    build_bilateral(nc, biases)
    nc.compile()
    return nc


def kernel(images, spatial_kernel):
    images = np.asarray(images, dtype=np.float32)
    spatial_kernel = np.asarray(spatial_kernel, dtype=np.float32)
    nc = make_program(spatial_kernel)
    shifts = _shift_mats()
    in_maps = [
        {"images": images[i * B_SH : (i + 1) * B_SH], "shifts": shifts}
        for i in range(N_CORES)
    ]
    res = run_bass_kernel_spmd(nc, in_maps, core_ids=list(range(N_CORES)))
    return np.concatenate([res.results[i]["out"] for i in range(N_CORES)], axis=0)



# revision 23
# speedup vs baseline: 1.2884x; 1.2884x over previous
"""Bilateral anti-alias filter on Trainium2, 8-core data parallel.

Full inputs: images [16,3,512,512] f32, spatial_kernel [5,5] f32.
Shards the batch over 8 NeuronCores (2 images each), runs a Bass/Tile
kernel per core, gathers the full output.

Math (per pixel, K=5, sigma_i=0.1), using pair symmetry over the 12
offsets t=(di,dj) with di>=0 lexicographically positive:

  d_t = p(x+t) - p(x)
  e_t = exp(-50 d_t^2)  computed as Derivative_Erf(sqrt(50) d)*sqrt(pi)/2
  u_t = e_t * d_t
  num(x) = p(x)*den(x) + sum_t [s+_t u_t - s-_t shift_t(u_t)]
  den(x) = s0 + sum_t [s+_t e_t + s-_t shift_t(e_t)]
  out = p + pa / den,   pa = sum_t [s+_t u_t - s-_t shift_t(u_t)]

shift_t realized on the TensorEngine via banded lhsT matmuls into PSUM
(spatial weights folded into the lhsT values); the final p*den product
cancels in the division, so only one DVE product per pair is needed.
"""
import sys

sys.path.insert(0, "/opt/trn_rl_repo")

import math
import numpy as np
import ml_dtypes
from contextlib import ExitStack

import concourse.bass as bass
import concourse.tile as tile
from concourse import bacc, mybir
from concourse.bass_utils import run_bass_kernel_spmd

f32 = mybir.dt.float32
bf16 = mybir.dt.bfloat16
AF = mybir.ActivationFunctionType
Alu = mybir.AluOpType

N_CORES = 8
B_FULL, C, H, W = 16, 3, 512, 512
B_SH = B_FULL // N_CORES  # 2 images per core
KK = 5
PAD = KK // 2  # 2
SQ50 = float(np.sqrt(np.float32(50.0)))
C_DERF = 2.0 / math.sqrt(math.pi)  # Derivative_Erf(x) = C_DERF*exp(-x^2)
NOUT = 124  # output rows per band
NG = 128    # plane partitions (= NOUT + 4)
WB = W + 4  # 516: padded col buffer, tile col c <-> image col c-2
WIN = W + 2  # 514: per-pair plane window width

# 12 pairs (di, dj) with di >= 0, lexicographically positive
PAIRS = [
    (0, 1), (0, 2),
    (1, -2), (1, -1), (1, 0), (1, 1), (1, 2),
    (2, -2), (2, -1), (2, 0), (2, 1), (2, 2),
]
BATCHES = [PAIRS[0:4], PAIRS[4:8], PAIRS[8:12]]
NB = 4  # pairs per batch

# CoreSim has no Derivative_Erf: emit Square+Exp instead (same numerics,
# same tile structure) when simulating.
SIM_SAFE_DERF = False


def _jbase(dj):
    """Image col of plane-window col 0 (window covers jbase..jbase+513)."""
    return -2 if dj > 0 else 0


def _shift_mats(spatial):
    """Banded lhsT matrices [NG, n_mats, NOUT] bf16 with spatial weights
    (divided by C_DERF) folded in. Returns (array, {(di,dj,kind): idx})."""
    def L(k, scale):
        a = np.zeros((NG, NOUT), np.float64)
        for m in range(NOUT):
            a[m + k, m] = scale
        return a

    mats, idx = [], {}
    for (di, dj) in PAIRS:
        sp = float(spatial[2 + di, 2 + dj]) / C_DERF
        sm = float(spatial[2 - di, 2 - dj]) / C_DERF
        if dj == 0:
            idx[(di, dj, "den")] = len(mats)
            mats.append(L(2, sp) + L(2 - di, sm))
            idx[(di, dj, "num")] = len(mats)
            mats.append(L(2, sp) - L(2 - di, sm))
        else:
            idx[(di, dj, "A")] = len(mats)
            mats.append(L(2, sp))
            idx[(di, dj, "B")] = len(mats)
            mats.append(L(2 - di, sm))
            idx[(di, dj, "C")] = len(mats)
            mats.append(L(2 - di, -sm))
    arr = np.stack(mats, 1)  # [NG, n_mats, NOUT]
    return arr.astype(ml_dtypes.bfloat16), idx


N_MATS = 2 * 2 + 10 * 3  # 34


def _row_bands(h):
    bands = list(range(0, h - NOUT + 1, NOUT))
    if bands[-1] != h - NOUT:
        bands.append(h - NOUT)
    return bands


def _reflect_runs(v0, v1, h):
    """Split virtual row range [v0, v1] into runs of physical rows.
    Returns list of (p_offset, phys_start, count, step) with step +-1."""
    runs = []
    v = v0
    while v <= v1:
        if v < 0:
            e = min(-1, v1)
            runs.append((v - v0, -v, e - v + 1, -1))
            v = e + 1
        elif v >= h:
            e = v1
            runs.append((v - v0, 2 * h - 2 - v, e - v + 1, -1))
            v = e + 1
        else:
            e = min(h - 1, v1)
            runs.append((v - v0, v, e - v + 1, 1))
            v = e + 1
    return runs


def build_bilateral(nc, s0, mat_idx, h=H, w=W, b_sh=B_SH, c=C):
    """Emit the per-core program. s0 = spatial[2,2] (center weight)."""
    img_d = nc.dram_tensor("images", [b_sh, c, h, w], f32, kind="ExternalInput").ap()
    shifts_d = nc.dram_tensor(
        "shifts", [NG, N_MATS, NOUT], bf16, kind="ExternalInput"
    ).ap()
    out_d = nc.dram_tensor("out", [b_sh, c, h, w], f32, kind="ExternalOutput").ap()

    # const APs for activation biases (0.0 for derf, s0 for Ln)
    for val in sorted({0.0, float(s0), float(np.log(C_DERF))}):
        key = (f32, val)
        if key not in nc.const_aps.aps:
            t = nc.alloc_sbuf_tensor(f"cbias-{val}", [128, 1], f32)
            nc.gpsimd.memset(t.ap(), val)
            nc.const_aps.aps[key] = t.ap()
    nc.all_engine_barrier()

    bands = _row_bands(h)

    with tile.TileContext(nc) as tc, ExitStack() as ctx:
        consts = ctx.enter_context(tc.tile_pool(name="consts", bufs=1))
        imgs_f = ctx.enter_context(tc.tile_pool(name="imgs_f", bufs=2))
        imgs_b = ctx.enter_context(tc.tile_pool(name="imgs_b", bufs=2))
        dpool = ctx.enter_context(tc.tile_pool(name="dpool", bufs=2))
        gpool = ctx.enter_context(tc.tile_pool(name="gpool", bufs=2))
        upool = ctx.enter_context(tc.tile_pool(name="upool", bufs=2))
        finals = ctx.enter_context(
            tc.tile_pool(name="finals", bufs=1 if SIM_SAFE_DERF else 2)
        )
        psums = ctx.enter_context(tc.tile_pool(name="psums", bufs=1, space="PSUM"))

        shifts = consts.tile([NG, N_MATS, NOUT], bf16)
        nc.sync.dma_start(shifts[:], shifts_d[:])

        for bi in range(b_sh):
            for r0 in bands:
                # ---- load 3 row-shifted reflect-padded f32 image copies ----
                ifs = []
                for s in range(3):
                    t = imgs_f.tile([NG, c, WB], f32, tag=f"i{s}f")
                    refl_rows = []
                    for (po, ps, cnt, step) in _reflect_runs(
                        r0 - 2 + s, r0 - 2 + s + NG - 1, h
                    ):
                        if step == 1:
                            src = img_d[bi, :, ps : ps + cnt, :]
                            nc.sync.dma_start(
                                t[po : po + cnt, :, 2 : 2 + w],
                                src.rearrange("c r n -> r c n"),
                            )
                        else:
                            for k in range(cnt):
                                refl_rows.append((po + k, ps - k))
                    for (p_dst, phys) in refl_rows:
                        p_src = phys - (r0 - 2 + s)
                        nc.sync.dma_start(
                            t[p_dst : p_dst + 1, :, 2 : 2 + w],
                            t[p_src : p_src + 1, :, 2 : 2 + w],
                        )
                    # reflect pad cols (image cols -2,-1,512,513), tiny DVE
                    for (j, jsrc) in ((0, 4), (1, 3), (2 + w, w), (3 + w, w - 1)):
                        nc.vector.tensor_copy(
                            t[:, :, j : j + 1], t[:, :, jsrc : jsrc + 1]
                        )
                    ifs.append(t)

                # bf16 casts A (DVE for s=0, Pool for s=1,2);
                # B copies (A shifted 1 col, via DMA) for odd-dj alignment
                ibA, ibB = [], []
                for s in range(3):
                    a = imgs_b.tile([NG, c, WB], bf16, tag=f"i{s}bA")
                    nc.vector.tensor_copy(a[:], ifs[s][:])
                    ibA.append(a)
                    b = imgs_b.tile([NG, c, WB], bf16, tag=f"i{s}bB")
                    nc.sync.dma_start(b[:, :, 0 : WB - 1], a[:, :, 1:WB])
                    ibB.append(b)

                # ---- PSUM accumulators ----
                pw = psums.tile([NOUT, c, 512], f32, tag="pw")
                pa = psums.tile([NOUT, c, 512], f32, tag="pa")

                n_per_ch = 2 * 1 + 10 * 2  # matmuls per psum bank (channel)
                pw_cnt = [0] * c
                pa_cnt = [0] * c

                pool_subs = 0
                for bt, batch in enumerate(BATCHES):
                    d = dpool.tile([NG, NB * c, WIN], bf16, tag="d")
                    for sl, (di, dj) in enumerate(batch):
                        jb = _jbase(dj)
                        cen = ibA[0][:, :, 2 + jb : 2 + jb + WIN]
                        if dj % 2 == 0:
                            sh = ibA[di][:, :, 2 + jb + dj : 2 + jb + dj + WIN]
                        else:
                            sh = ibB[di][:, :, 1 + jb + dj : 1 + jb + dj + WIN]
                        dsl = d[:, sl * c : (sl + 1) * c, :]
                        nc.vector.tensor_tensor(dsl, sh, cen, Alu.subtract)
                    g = gpool.tile([NG, NB * c, WIN], bf16, tag="g")
                    if SIM_SAFE_DERF:
                        sq = gpool.tile([NG, NB * c, WIN], bf16, tag="sq")
                        nc.scalar.activation(
                            sq[:], d[:], AF.Square, bias=0.0, scale=SQ50
                        )
                        nc.scalar.activation(
                            g[:], sq[:], AF.Exp,
                            bias=float(np.log(C_DERF)), scale=-1.0,
                        )
                    else:
                        nc.scalar.activation(
                            g[:], d[:], AF.Derivative_Erf, bias=0.0, scale=SQ50
                        )
                    u = upool.tile([NG, NB * c, WIN], bf16, tag="u")
                    nc.vector.tensor_tensor(u[:], g[:], d[:], Alu.mult)

                    # ---- PE accumulation streams ----
                    # start/stop are per PSUM zero-region (= per channel bank)
                    def mm_pw(mat, rhs):
                        k = pw_cnt[rhs_ch]
                        nc.tensor.matmul(
                            pw[:, rhs_ch, :], mat, rhs,
                            start=k == 0, stop=k == n_per_ch - 1,
                        )
                        pw_cnt[rhs_ch] = k + 1

                    def mm_pa(mat, rhs):
                        k = pa_cnt[rhs_ch]
                        nc.tensor.matmul(
                            pa[:, rhs_ch, :], mat, rhs,
                            start=k == 0, stop=k == n_per_ch - 1,
                        )
                        pa_cnt[rhs_ch] = k + 1

                    for sl, (di, dj) in enumerate(batch):
                        jb = _jbase(dj)
                        od = -jb            # direct window offset in plane
                        os_ = -jb - dj      # shifted window offset
                        if dj == 0:
                            for rhs_ch in range(c):
                                mm_pw(shifts[:, mat_idx[(di, dj, "den")], :],
                                      g[:, sl * c + rhs_ch, od : od + 512])
                            for rhs_ch in range(c):
                                mm_pa(shifts[:, mat_idx[(di, dj, "num")], :],
                                      u[:, sl * c + rhs_ch, od : od + 512])
                        else:
                            for rhs_ch in range(c):
                                mm_pw(shifts[:, mat_idx[(di, dj, "A")], :],
                                      g[:, sl * c + rhs_ch, od : od + 512])
                            for rhs_ch in range(c):
                                mm_pa(shifts[:, mat_idx[(di, dj, "A")], :],
                                      u[:, sl * c + rhs_ch, od : od + 512])
                            for rhs_ch in range(c):
                                mm_pw(shifts[:, mat_idx[(di, dj, "B")], :],
                                      g[:, sl * c + rhs_ch, os_ : os_ + 512])
                            for rhs_ch in range(c):
                                mm_pa(shifts[:, mat_idx[(di, dj, "C")], :],
                                      u[:, sl * c + rhs_ch, os_ : os_ + 512])

                # ---- finals: out = p + pa / (pw + s0) ----
                pacopy = finals.tile([NOUT, c, 512], bf16, tag="pac")
                nc.vector.tensor_copy(pacopy[:], pa[:])
                lnv = finals.tile([NOUT, c, 512], f32, tag="lnv")
                nc.scalar.activation(lnv[:], pw[:], AF.Ln, bias=float(s0))
                rec = finals.tile([NOUT, c, 512], bf16, tag="rec")
                nc.scalar.activation(rec[:], lnv[:], AF.Exp, scale=-1.0)
                res = finals.tile([NOUT, c, 512], bf16, tag="res")
                nc.vector.tensor_tensor(res[:], pacopy[:], rec[:], Alu.mult)
                outp = finals.tile([NOUT, c, 512], f32, tag="outp")
                nc.vector.tensor_tensor(
                    outp[:], res[:], ifs[2][0:NOUT, :, 2 : 2 + w], Alu.add
                )
                oo = 0 if r0 == bands[0] else max(0, prev_end - r0)
                nc.sync.dma_start(
                    out_d[bi, :, r0 + oo : r0 + NOUT, :].rearrange("c r n -> r c n"),
                    outp[oo:NOUT],
                )
                prev_end = r0 + NOUT
    return nc


def _restrict_act_tables():
    """Steer the activation-table chooser so per-band table swaps stay at
    2 (derf set <-> ln/exp set): keep every set (indices into
    act_info.json must be preserved) but strip Exp/Ln/Derivative_Erf
    membership from all other sets so they can't be chosen for them."""
    import concourse.bacc as cbacc

    if getattr(cbacc.get_activation_tables, "_bilateral_patched", False):
        return
    orig = cbacc.get_activation_tables
    keep = {"erf_derivative", "natural_log_exp_and_others"}
    strip = {AF.Exp, AF.Ln, AF.Derivative_Erf}

    def patched(arch):
        tabs = orig(arch)
        return {
            k: (set(v) if k in keep else set(v) - strip)
            for k, v in tabs.items()
        }

    patched._bilateral_patched = True
    cbacc.get_activation_tables = patched


def make_program(spatial_kernel):
    spatial_kernel = np.asarray(spatial_kernel, dtype=np.float32)
    mats, mat_idx = _shift_mats(spatial_kernel)
    s0 = float(spatial_kernel[2, 2])
    _restrict_act_tables()
    nc = bacc.Bacc("TRN2", target_bir_lowering=False, debug=False)
    build_bilateral(nc, s0, mat_idx)
    nc.compile()
    return nc, mats


def kernel(images, spatial_kernel):
    images = np.asarray(images, dtype=np.float32)
    spatial_kernel = np.asarray(spatial_kernel, dtype=np.float32)
    nc, mats = make_program(spatial_kernel)
    in_maps = [
        {"images": images[i * B_SH : (i + 1) * B_SH], "shifts": mats}
        for i in range(N_CORES)
    ]
    res = run_bass_kernel_spmd(nc, in_maps, core_ids=list(range(N_CORES)))
    return np.concatenate([res.results[i]["out"] for i in range(N_CORES)], axis=0)
